# revision 29
# baseline (speedup 1.0000x reference)
import sys

sys.path.insert(0, "/opt/trn_rl_repo")

from contextlib import ExitStack

import ml_dtypes
import numpy as np

from concourse import bass, bass_isa, mybir, tile
from concourse.bass_utils import run_bass_kernel_spmd
from concourse.vector_clock import ScopedClock


def _patched_drain_and_barrier(self, tick_clock, wait_clock):
    # Workaround: this compiler rejects a drain carrying >1 sem wait
    # ([NCC_INLA001]); split extra waits onto single-wait nops.
    drain_inst = self.nc.sync.drain()
    wait_clock.add_sem_waits(
        drain_inst.ins, ScopedClock({None: tick_clock.global_clock})
    )
    si = drain_inst.ins.sync_info
    waits = list(si.on_wait) if si and si.on_wait else []
    if len(waits) > 1:
        drain_inst.ins.sync_info = mybir.SyncInfo(
            on_wait=[waits[0]], on_update=list(si.on_update or [])
        )
        for w in waits[1:]:
            nop = self.nc.sync.nop(nofuse=True)
            nop.ins.sync_info = mybir.SyncInfo(on_wait=[w], on_update=[])
    self.nc.all_engine_barrier()
    popped = self.nc._tile_sem_poison_stack.pop()
    assert popped is self._sem_poison
    self.nc.clear_and_free_semaphores(list(self.sems.allocated().values()))
    self.nc.all_engine_barrier()


tile.TileContext._drain_and_barrier = _patched_drain_and_barrier


def _split_excess_waits(nc, limit=1):
    # Workaround: this compiler allows only one sem wait on several
    # instruction encodings; move extra waits onto same-engine nops.
    eng_map = {
        mybir.EngineType.PE: nc.tensor,
        mybir.EngineType.Activation: nc.scalar,
        mybir.EngineType.DVE: nc.vector,
        mybir.EngineType.Pool: nc.gpsimd,
        mybir.EngineType.SP: nc.sync,
    }
    for blk in nc.cur_f.blocks:
        orig = list(blk.instructions)
        out = []
        for ins in orig:
            si = ins.sync_info
            waits = list(si.on_wait) if si and si.on_wait else []
            eng = eng_map.get(ins.engine)
            if len(waits) > limit and eng is not None:
                extra, keep = waits[:-limit], waits[-limit:]
                for w in extra:
                    nop = eng.nop(nofuse=True).ins
                    tail = nc.cur_f.blocks[-1].instructions
                    assert tail[-1] is nop
                    tail.pop()
                    nop.sync_info = mybir.SyncInfo(on_wait=[w], on_update=[])
                    out.append(nop)
                ins.sync_info = mybir.SyncInfo(
                    on_wait=keep, on_update=list(si.on_update or [])
                )
            out.append(ins)
        blk.instructions[:] = out

bf16 = ml_dtypes.bfloat16
BF = bass.mybir.dt.bfloat16
F32 = bass.mybir.dt.float32
AF = mybir.ActivationFunctionType
ALU = mybir.AluOpType

B, S, E, H, D = 2, 2048, 2048, 16, 128
BS = B * S
NCORES = 8
HPC = H // NCORES  # heads per core
DC = HPC * D  # per-core head-dim width (256)
SCALE = 1.0 / float(np.sqrt(D))

TRACE = False
LAST_RESULTS = None
_NC_CACHE = None


def _build():
    nc = bass.Bass()
    xT = nc.declare_dram_parameter("xT", (E, BS), BF, isOutput=False)
    # weights pre-packed on host to [128, ...] so each is ONE contiguous DMA
    wq3 = nc.declare_dram_parameter("wq3", (128, 16 * DC), BF, isOutput=False)
    wk3 = nc.declare_dram_parameter("wk3", (128, 16 * DC), BF, isOutput=False)
    wv3 = nc.declare_dram_parameter("wv3", (128, 16 * DC), BF, isOutput=False)
    wo3 = nc.declare_dram_parameter("wo3", (128, HPC * E), BF, isOutput=False)
    bqd = nc.declare_dram_parameter("bq", (128, HPC), F32, isOutput=False)
    mscd = nc.declare_dram_parameter("misc", (128, 2 * 128), BF, isOutput=False)
    yd = nc.declare_dram_parameter("y", (BS, E), BF, isOutput=True)

    with ExitStack() as ctx:
        tc = ctx.enter_context(tile.TileContext(nc))
        wp = ctx.enter_context(tc.tile_pool(name="wp", bufs=1))
        bp = ctx.enter_context(tc.tile_pool(name="bp", bufs=1))
        pp = ctx.enter_context(tc.tile_pool(name="pp", bufs=10))
        dp = ctx.enter_context(tc.tile_pool(name="dp", bufs=2))
        yp = ctx.enter_context(tc.tile_pool(name="yp", bufs=8))
        ps = ctx.enter_context(tc.tile_pool(name="ps", bufs=1, space="PSUM"))

        wq_sb = wp.tile([128, 16, DC], BF)
        wk_sb = wp.tile([128, 16, DC], BF)
        wv_sb = wp.tile([128, 16, DC], BF)
        wo_sb = wp.tile([128, HPC, E], BF)
        bq_sb = wp.tile([128, HPC], F32)
        msc_sb = wp.tile([128, 2, 128], BF)
        msk_sb = msc_sb[:, 0, :]  # 0/1 lower-tri (k<=q) [k,q]
        onk_sb = msc_sb[:, 1, :]  # ones

        for b in range(B):
            s0 = b * S
            x_sb = bp.tile([128, 16, S], BF, tag="x")
            def _wchunk(dst, src, tc4):
                cs = slice(tc4 * 4 * DC, (tc4 + 1) * 4 * DC)
                nc.sync.dma_start(
                    dst[:, tc4 * 4 : (tc4 + 1) * 4, :],
                    src[:, cs].rearrange("p (t d) -> p t d", t=4),
                )

            # x in column halves: j-blocks 0/1 only depend on the first
            # 4MB of x, so the j0 wave starts ~15us earlier. Weight
            # t-chunks are interleaved in consumption order (transfers
            # complete roughly in issue order at aggregate bandwidth).
            if b == 0:
                _wchunk(wq_sb, wq3, 0)
                _wchunk(wk_sb, wk3, 0)
                _wchunk(wv_sb, wv3, 0)
            for half in range(2):
                hs = slice(half * 1024, (half + 1) * 1024)
                for t in range(16):
                    nc.sync.dma_start(
                        x_sb[:, t, hs],
                        xT[
                            t * 128 : (t + 1) * 128,
                            s0 + half * 1024 : s0 + (half + 1) * 1024,
                        ],
                    )
                    if b == 0 and half == 0 and t in (2, 4, 6):
                        _wchunk(wq_sb, wq3, t // 2)
                        _wchunk(wk_sb, wk3, t // 2)
                        _wchunk(wv_sb, wv3, t // 2)
                    if b == 0 and half == 0 and t == 8:
                        nc.sync.dma_start(bq_sb[:], bqd[:])
                        nc.sync.dma_start(
                            msc_sb[:], mscd[:].rearrange("p (a k) -> p a k", a=2)
                        )
                    if b == 0 and half == 1 and t == 0:
                        nc.sync.dma_start(
                            wo_sb[:], wo3[:].rearrange("p (h e) -> p h e", h=HPC)
                        )
            qT_sb = bp.tile([128, HPC, S], BF, tag="qT")
            kT_sb = bp.tile([128, HPC, S], BF, tag="kT")
            v_sb = bp.tile([128, 16, DC], BF, tag="v")
            ctxN_sb = bp.tile([128, HPC, S], BF, tag="cN", bufs=2)

            tags = [("pr", 2), ("pr", 2), ("sc", 3), ("sc", 3),
                    ("sc", 3), ("cx", 3), ("cx", 3), ("cx", 3)]

            def proj_jblock(j, wave):
                # q/k/v projections for output column block j (contract E in
                # 16 chunks of 128). wave=True: all 8 chains advance in
                # parallel across the 8 PSUM banks, inner loop over t, so PE
                # paces the x DMA wave. wave=False: chain-major so psum
                # drains stagger and banks free progressively.
                # k bias is skipped entirely: softmax is invariant to the
                # per-query constant q·bk it would add to every score row.
                js = slice(j * 512, (j + 1) * 512)
                qk_ps = []
                for m in range(HPC):
                    tg, bf = tags[2 * m], tags[2 * m + 1]
                    qm_ps = ps.tile([128, 512], F32, tag=tg[0], bufs=tg[1], name=f"q{m}")
                    km_ps = ps.tile([128, 512], F32, tag=bf[0], bufs=bf[1], name=f"k{m}")
                    qk_ps.append(qm_ps)
                    qk_ps.append(km_ps)
                v_pss = []
                for si in range(4):
                    vs_ps = ps.tile(
                        [128, DC], F32, tag=tags[4 + si][0], bufs=tags[4 + si][1],
                        name=f"v{si}",
                    )
                    v_pss.append(vs_ps)

                def qk_mm(m, t):
                    nc.tensor.matmul(
                        qk_ps[2 * m][:],
                        wq_sb[:, t, m * 128 : (m + 1) * 128],
                        x_sb[:, t, js],
                        start=(t == 0),
                        stop=(t == 15),
                    )
                    nc.tensor.matmul(
                        qk_ps[2 * m + 1][:],
                        wk_sb[:, t, m * 128 : (m + 1) * 128],
                        x_sb[:, t, js],
                        start=(t == 0),
                        stop=(t == 15),
                    )

                def v_mm(si, t):
                    nc.tensor.matmul(
                        v_pss[si][:],
                        x_sb[:, t, (4 * j + si) * 128 : (4 * j + si + 1) * 128],
                        wv_sb[:, t, :],
                        start=(t == 0),
                        stop=(t == 15),
                    )

                def qk_drain(m):
                    nc.scalar.activation(
                        qT_sb[:, m, js], qk_ps[2 * m][:], AF.Identity,
                        bias=bq_sb[:, m : m + 1],
                    )
                    nc.vector.tensor_copy(kT_sb[:, m, js], qk_ps[2 * m + 1][:])

                def v_drain(si):
                    if si == 3:
                        nc.scalar.copy(v_sb[:, 4 * j + si, :], v_pss[si][:])
                    else:
                        nc.vector.tensor_copy(v_sb[:, 4 * j + si, :], v_pss[si][:])

                if wave:
                    for t in range(16):
                        for m in range(HPC):
                            qk_mm(m, t)
                        for si in range(4):
                            v_mm(si, t)
                    for m in range(HPC):
                        qk_drain(m)
                    for si in range(4):
                        v_drain(si)
                else:
                    for m in range(HPC):
                        for t in range(16):
                            qk_mm(m, t)
                        qk_drain(m)
                    for si in range(4):
                        for t in range(16):
                            v_mm(si, t)
                        v_drain(si)

            def attn_pair(qb):
                # causal attention for one query-block, both heads with
                # their chunk streams interleaved: one pipeline fill/drain
                # per pair, and den/ctx trail exp by a full chunk-pair so
                # PE never waits on ACT. Scores kept transposed [k, q];
                # valid-column trim: chunk kc only contributes to queries
                # q >= kc*128, so all work runs on the [off:512] slice.
                q0 = qb * 512
                kmax = 4 * qb + 4
                pts = {0: [], 1: []}
                den_ps = {}
                ctx_ps = {}
                for h in range(HPC):
                    dh_ps = ps.tile([128, 512], F32, tag="pr", bufs=2, name=f"den{h}")
                    ch_ps = ps.tile([128, 512], F32, tag="cx", bufs=3, name=f"ctx{h}")
                    den_ps[h] = dh_ps
                    ctx_ps[h] = ch_ps
                LAG = 2
                for kc in range(kmax + LAG):
                    if kc < kmax:
                        diag = kc - 4 * qb
                        off = max(0, 128 * diag)
                        for h in range(HPC):
                            sc_ps = ps.tile([128, 512], F32, tag="sc", bufs=3)
                            nc.tensor.matmul(
                                sc_ps[:, off:512],
                                kT_sb[:, h, kc * 128 : (kc + 1) * 128],
                                qT_sb[:, h, q0 + off : q0 + 512],
                                start=True,
                                stop=True,
                            )
                            p_t = pp.tile([128, 512], BF)
                            nc.scalar.activation(
                                p_t[:, off:512], sc_ps[:, off:512], AF.Exp
                            )
                            if diag >= 0:
                                nc.vector.tensor_tensor(
                                    p_t[:, off : off + 128],
                                    p_t[:, off : off + 128],
                                    msk_sb,
                                    ALU.mult,
                                )
                            pts[h].append((p_t, off))
                    j = kc - LAG
                    if j >= 0:
                        for h in range(HPC):
                            pj, oj = pts[h][j]
                            nc.tensor.matmul(
                                den_ps[h][:, oj:512],
                                onk_sb,
                                pj[:, oj:512],
                                start=(j == 0),
                                stop=(j == kmax - 1),
                                skip_group_check=True,
                            )
                            nc.tensor.matmul(
                                ctx_ps[h][:, oj:512],
                                v_sb[:, j, h * 128 : (h + 1) * 128],
                                pj[:, oj:512],
                                start=(j == 0),
                                stop=(j == kmax - 1),
                                skip_group_check=True,
                            )
                for h in range(HPC):
                    recb_sb = dp.tile([128, 512], F32, tag="recb", bufs=2)
                    nc.vector.reciprocal(recb_sb[:], den_ps[h][:])
                    nc.vector.tensor_tensor(
                        ctxN_sb[:, h, q0 : q0 + 512], ctx_ps[h][:], recb_sb[:],
                        ALU.mult,
                    )

            ytags = [("pr", 2), ("sc", 3), ("cx", 3)]

            def outproj_group(g):
                # output projection for query chunks 4g..4g+3 (contract
                # per-core d=256 in 2 head chunks). y_ps rotates across all
                # 3 psum tags; drains alternate DVE/ACT so copies never
                # pace the PE.
                for qc in range(4 * g, 4 * g + 4):
                    for ep in range(2):
                        y_t = yp.tile([128, 1024], BF)
                        for ei in range(2):
                            eb = ep * 2 + ei
                            n = qc * 4 + eb
                            tg = ytags[n % 3]
                            y_ps = ps.tile([128, 512], F32, tag=tg[0], bufs=tg[1])
                            nc.tensor.matmul(
                                y_ps[:],
                                ctxN_sb[:, 0, qc * 128 : (qc + 1) * 128],
                                wo_sb[:, 0, eb * 512 : (eb + 1) * 512],
                                start=True,
                                stop=False,
                            )
                            nc.tensor.matmul(
                                y_ps[:],
                                ctxN_sb[:, 1, qc * 128 : (qc + 1) * 128],
                                wo_sb[:, 1, eb * 512 : (eb + 1) * 512],
                                start=False,
                                stop=True,
                            )
                            if n % 2 == 0:
                                nc.vector.tensor_copy(
                                    y_t[:, ei * 512 : (ei + 1) * 512], y_ps[:]
                                )
                            else:
                                nc.scalar.copy(
                                    y_t[:, ei * 512 : (ei + 1) * 512], y_ps[:]
                                )
                        nc.sync.dma_start(
                            yd[
                                s0 + qc * 128 : s0 + (qc + 1) * 128,
                                ep * 1024 : (ep + 1) * 1024,
                            ],
                            y_t[:],
                        )

            # Phase-separated emission: wave-major j-blocks (PE paces the
            # x DMA), last block chain-major so its psum drains stagger
            # and attention's first score matmuls find free banks.
            proj_jblock(0, wave=True)
            proj_jblock(1, wave=True)
            proj_jblock(2, wave=True)
            proj_jblock(3, wave=True)
            for qb in range(4):
                attn_pair(qb)
            for g in range(4):
                outproj_group(g)
    _split_excess_waits(nc)
    return nc


def _pack_w(w):
    # [E, DC] -> [128, 16*DC] so row p holds w[t*128+p, :] for t=0..15
    return np.ascontiguousarray(
        w.reshape(16, 128, DC).transpose(1, 0, 2).reshape(128, 16 * DC)
    )


def kernel(**inputs):
    global LAST_RESULTS, _NC_CACHE
    x = np.asarray(inputs["x"], np.float32)
    Wq = np.asarray(inputs["Wq"], np.float32)
    bq = np.asarray(inputs["bq"], np.float32)
    Wk = np.asarray(inputs["Wk"], np.float32)
    bk = np.asarray(inputs["bk"], np.float32)
    Wv = np.asarray(inputs["Wv"], np.float32)
    bv = np.asarray(inputs["bv"], np.float32)
    Wo = np.asarray(inputs["Wo"], np.float32)
    bo = np.asarray(inputs["bo"], np.float32)

    xT = np.ascontiguousarray(x.reshape(BS, E).T).astype(bf16)
    kk = np.arange(128)[:, None]
    qq = np.arange(128)[None, :]
    misc = np.concatenate(
        [
            np.where(kk <= qq, 1.0, 0.0),
            np.ones((128, 128)),
        ],
        axis=1,
    ).astype(bf16)

    in_maps = []
    for c in range(NCORES):
        dsl = slice(c * DC, (c + 1) * DC)
        in_maps.append(
            {
                "xT": xT,
                "wq3": _pack_w((Wq[dsl].T * SCALE).astype(bf16)),
                "wk3": _pack_w(Wk[dsl].T.astype(bf16)),
                "wv3": _pack_w(Wv[dsl].T.astype(bf16)),
                "wo3": np.ascontiguousarray(
                    Wo[:, dsl].T.astype(bf16)
                    .reshape(HPC, 128, E)
                    .transpose(1, 0, 2)
                    .reshape(128, HPC * E)
                ),
                "bq": np.ascontiguousarray(
                    (bq[dsl] * SCALE).astype(np.float32).reshape(HPC, 128).T
                ),
                "misc": misc,
            }
        )

    if _NC_CACHE is None:
        _NC_CACHE = _build()
    res = run_bass_kernel_spmd(_NC_CACHE, in_maps, core_ids=list(range(NCORES)), trace=TRACE)
    LAST_RESULTS = res

    acc = None
    for r in res.results:
        yc = np.asarray(r["y"], np.float32)
        acc = yc if acc is None else acc + yc
    bo_eff = bo + bv @ Wo.T
    acc += bo_eff[None, :]
    return acc.reshape(B, S, E).astype(np.float32)


# revision 31
# speedup vs baseline: 1.1291x; 1.1291x over previous
import sys

sys.path.insert(0, "/opt/trn_rl_repo")

from contextlib import ExitStack

import ml_dtypes
import numpy as np

from concourse import bass, bass_isa, mybir, tile
from concourse.bass_utils import run_bass_kernel_spmd
from concourse.vector_clock import ScopedClock


def _patched_drain_and_barrier(self, tick_clock, wait_clock):
    # Workaround: this compiler rejects a drain carrying >1 sem wait
    # ([NCC_INLA001]); split extra waits onto single-wait nops.
    drain_inst = self.nc.sync.drain()
    wait_clock.add_sem_waits(
        drain_inst.ins, ScopedClock({None: tick_clock.global_clock})
    )
    si = drain_inst.ins.sync_info
    waits = list(si.on_wait) if si and si.on_wait else []
    if len(waits) > 1:
        drain_inst.ins.sync_info = mybir.SyncInfo(
            on_wait=[waits[0]], on_update=list(si.on_update or [])
        )
        for w in waits[1:]:
            nop = self.nc.sync.nop(nofuse=True)
            nop.ins.sync_info = mybir.SyncInfo(on_wait=[w], on_update=[])
    self.nc.all_engine_barrier()
    popped = self.nc._tile_sem_poison_stack.pop()
    assert popped is self._sem_poison
    self.nc.clear_and_free_semaphores(list(self.sems.allocated().values()))
    self.nc.all_engine_barrier()


tile.TileContext._drain_and_barrier = _patched_drain_and_barrier


def _split_excess_waits(nc, limit=1):
    # Workaround: this compiler allows only one sem wait on several
    # instruction encodings; move extra waits onto same-engine nops.
    eng_map = {
        mybir.EngineType.PE: nc.tensor,
        mybir.EngineType.Activation: nc.scalar,
        mybir.EngineType.DVE: nc.vector,
        mybir.EngineType.Pool: nc.gpsimd,
        mybir.EngineType.SP: nc.sync,
    }
    for blk in nc.cur_f.blocks:
        orig = list(blk.instructions)
        out = []
        for ins in orig:
            si = ins.sync_info
            waits = list(si.on_wait) if si and si.on_wait else []
            eng = eng_map.get(ins.engine)
            if len(waits) > limit and eng is not None:
                extra, keep = waits[:-limit], waits[-limit:]
                for w in extra:
                    nop = eng.nop(nofuse=True).ins
                    tail = nc.cur_f.blocks[-1].instructions
                    assert tail[-1] is nop
                    tail.pop()
                    nop.sync_info = mybir.SyncInfo(on_wait=[w], on_update=[])
                    out.append(nop)
                ins.sync_info = mybir.SyncInfo(
                    on_wait=keep, on_update=list(si.on_update or [])
                )
            out.append(ins)
        blk.instructions[:] = out

bf16 = ml_dtypes.bfloat16
BF = bass.mybir.dt.bfloat16
F32 = bass.mybir.dt.float32
AF = mybir.ActivationFunctionType
ALU = mybir.AluOpType

B, S, E, H, D = 2, 2048, 2048, 16, 128
BS = B * S
NCORES = 8
HPC = H // NCORES  # heads per core
DC = HPC * D  # per-core head-dim width (256)
SCALE = 1.0 / float(np.sqrt(D))

TRACE = False
LAST_RESULTS = None
_NC_CACHE = None


def _build():
    nc = bass.Bass()
    xT = nc.declare_dram_parameter("xT", (E, BS), BF, isOutput=False)
    # weights pre-packed on host to [128, ...] so each is ONE contiguous DMA
    wq3 = nc.declare_dram_parameter("wq3", (128, 16 * DC), BF, isOutput=False)
    wk3 = nc.declare_dram_parameter("wk3", (128, 16 * DC), BF, isOutput=False)
    wv3 = nc.declare_dram_parameter("wv3", (128, 16 * DC), BF, isOutput=False)
    wo3 = nc.declare_dram_parameter("wo3", (128, HPC * E), BF, isOutput=False)
    bqd = nc.declare_dram_parameter("bq", (128, HPC), F32, isOutput=False)
    mscd = nc.declare_dram_parameter("misc", (128, 2 * 128), BF, isOutput=False)
    yd = nc.declare_dram_parameter("y", (BS, E), BF, isOutput=True)

    with ExitStack() as ctx:
        tc = ctx.enter_context(tile.TileContext(nc))
        wp = ctx.enter_context(tc.tile_pool(name="wp", bufs=1))
        bp = ctx.enter_context(tc.tile_pool(name="bp", bufs=1))
        pp = ctx.enter_context(tc.tile_pool(name="pp", bufs=10))
        dp = ctx.enter_context(tc.tile_pool(name="dp", bufs=2))
        yp = ctx.enter_context(tc.tile_pool(name="yp", bufs=8))
        ps = ctx.enter_context(tc.tile_pool(name="ps", bufs=1, space="PSUM"))

        wq_sb = wp.tile([128, 16, DC], BF)
        wk_sb = wp.tile([128, 16, DC], BF)
        wv_sb = wp.tile([128, 16, DC], BF)
        wo_sb = wp.tile([128, HPC, E], BF)
        bq_sb = wp.tile([128, HPC], F32)
        msc_sb = wp.tile([128, 2, 128], BF)
        msk_sb = msc_sb[:, 0, :]  # 0/1 lower-tri (k<=q) [k,q]
        onk_sb = msc_sb[:, 1, :]  # ones

        for b in range(B):
            s0 = b * S
            x_sb = bp.tile([128, 16, S], BF, tag="x")
            def _wchunk(dst, src, tc4):
                cs = slice(tc4 * 4 * DC, (tc4 + 1) * 4 * DC)
                nc.sync.dma_start(
                    dst[:, tc4 * 4 : (tc4 + 1) * 4, :],
                    src[:, cs].rearrange("p (t d) -> p t d", t=4),
                )

            # x in column halves: j-blocks 0/1 only depend on the first
            # 4MB of x, so the j0 wave starts ~15us earlier. Weight
            # t-chunks are interleaved in consumption order (transfers
            # complete roughly in issue order at aggregate bandwidth).
            if b == 0:
                _wchunk(wq_sb, wq3, 0)
                _wchunk(wk_sb, wk3, 0)
                _wchunk(wv_sb, wv3, 0)
            for half in range(2):
                hs = slice(half * 1024, (half + 1) * 1024)
                for t in range(16):
                    nc.sync.dma_start(
                        x_sb[:, t, hs],
                        xT[
                            t * 128 : (t + 1) * 128,
                            s0 + half * 1024 : s0 + (half + 1) * 1024,
                        ],
                    )
                    if b == 0 and half == 0 and t in (2, 4, 6):
                        _wchunk(wq_sb, wq3, t // 2)
                        _wchunk(wk_sb, wk3, t // 2)
                        _wchunk(wv_sb, wv3, t // 2)
                    if b == 0 and half == 0 and t == 8:
                        nc.sync.dma_start(bq_sb[:], bqd[:])
                        nc.sync.dma_start(
                            msc_sb[:], mscd[:].rearrange("p (a k) -> p a k", a=2)
                        )
                    if b == 0 and half == 1 and t == 0:
                        nc.sync.dma_start(
                            wo_sb[:], wo3[:].rearrange("p (h e) -> p h e", h=HPC)
                        )
            qT_sb = bp.tile([128, HPC, S], BF, tag="qT")
            kT_sb = bp.tile([128, HPC, S], BF, tag="kT")
            v_sb = bp.tile([128, 16, DC], BF, tag="v")
            ctxN_sb = bp.tile([128, HPC, S], BF, tag="cN", bufs=2)

            tags = [("pr", 2), ("pr", 2), ("sc", 3), ("sc", 3),
                    ("sc", 3), ("cx", 3), ("cx", 3), ("cx", 3)]

            def proj_jblock(j, wave):
                # q/k/v projections for output column block j (contract E in
                # 16 chunks of 128). wave=True: all 8 chains advance in
                # parallel across the 8 PSUM banks, inner loop over t, so PE
                # paces the x DMA wave. wave=False: chain-major so psum
                # drains stagger and banks free progressively.
                # k bias is skipped entirely: softmax is invariant to the
                # per-query constant q·bk it would add to every score row.
                js = slice(j * 512, (j + 1) * 512)
                qk_ps = []
                for m in range(HPC):
                    tg, bf = tags[2 * m], tags[2 * m + 1]
                    qm_ps = ps.tile([128, 512], F32, tag=tg[0], bufs=tg[1], name=f"q{m}")
                    km_ps = ps.tile([128, 512], F32, tag=bf[0], bufs=bf[1], name=f"k{m}")
                    qk_ps.append(qm_ps)
                    qk_ps.append(km_ps)
                v_pss = []
                for si in range(4):
                    vs_ps = ps.tile(
                        [128, DC], F32, tag=tags[4 + si][0], bufs=tags[4 + si][1],
                        name=f"v{si}",
                    )
                    v_pss.append(vs_ps)

                def qk_mm(m, t):
                    nc.tensor.matmul(
                        qk_ps[2 * m][:],
                        wq_sb[:, t, m * 128 : (m + 1) * 128],
                        x_sb[:, t, js],
                        start=(t == 0),
                        stop=(t == 15),
                    )
                    nc.tensor.matmul(
                        qk_ps[2 * m + 1][:],
                        wk_sb[:, t, m * 128 : (m + 1) * 128],
                        x_sb[:, t, js],
                        start=(t == 0),
                        stop=(t == 15),
                    )

                def v_mm(si, t):
                    nc.tensor.matmul(
                        v_pss[si][:],
                        x_sb[:, t, (4 * j + si) * 128 : (4 * j + si + 1) * 128],
                        wv_sb[:, t, :],
                        start=(t == 0),
                        stop=(t == 15),
                    )

                def qk_drain(m):
                    nc.scalar.activation(
                        qT_sb[:, m, js], qk_ps[2 * m][:], AF.Identity,
                        bias=bq_sb[:, m : m + 1],
                    )
                    nc.vector.tensor_copy(kT_sb[:, m, js], qk_ps[2 * m + 1][:])

                def v_drain(si):
                    if si == 3:
                        nc.scalar.copy(v_sb[:, 4 * j + si, :], v_pss[si][:])
                    else:
                        nc.vector.tensor_copy(v_sb[:, 4 * j + si, :], v_pss[si][:])

                if wave:
                    for t in range(16):
                        for m in range(HPC):
                            qk_mm(m, t)
                        for si in range(4):
                            v_mm(si, t)
                    for m in range(HPC):
                        qk_drain(m)
                    for si in range(4):
                        v_drain(si)
                else:
                    for m in range(HPC):
                        for t in range(16):
                            qk_mm(m, t)
                        qk_drain(m)
                    for si in range(4):
                        for t in range(16):
                            v_mm(si, t)
                        v_drain(si)

            def attn_unit(qb, h, LAG=2):
                # causal attention for one (query-block, head); scores kept
                # transposed [k, q]. Valid-column trim: chunk kc only
                # contributes to queries q >= kc*128, so all work runs on
                # the [off:512] column slice.
                q0 = qb * 512
                kmax = 4 * qb + 4
                hd = slice(h * 128, (h + 1) * 128)
                pts = []
                den_ps = ps.tile([128, 512], F32, tag="pr", bufs=2)
                ctx_ps = ps.tile([128, 512], F32, tag="cx", bufs=3)
                # interleave den/ctx accumulation (lagging LAG tiles)
                # between score matmuls so PE never waits on ACT exp
                for kc in range(kmax + LAG):
                    if kc < kmax:
                        diag = kc - 4 * qb
                        off = max(0, 128 * diag)
                        sc_ps = ps.tile([128, 512], F32, tag="sc", bufs=3)
                        nc.tensor.matmul(
                            sc_ps[:, off:512],
                            kT_sb[:, h, kc * 128 : (kc + 1) * 128],
                            qT_sb[:, h, q0 + off : q0 + 512],
                            start=True,
                            stop=True,
                        )
                        p_t = pp.tile([128, 512], BF)
                        nc.scalar.activation(
                            p_t[:, off:512], sc_ps[:, off:512], AF.Exp
                        )
                        if diag >= 0:
                            nc.vector.tensor_tensor(
                                p_t[:, off : off + 128],
                                p_t[:, off : off + 128],
                                msk_sb,
                                ALU.mult,
                            )
                        pts.append((p_t, off))
                    j = kc - LAG
                    if j >= 0:
                        pj, oj = pts[j]
                        nc.tensor.matmul(
                            den_ps[:, oj:512],
                            onk_sb,
                            pj[:, oj:512],
                            start=(j == 0),
                            stop=(j == kmax - 1),
                            skip_group_check=True,
                        )
                        nc.tensor.matmul(
                            ctx_ps[:, oj:512],
                            v_sb[:, j, hd],
                            pj[:, oj:512],
                            start=(j == 0),
                            stop=(j == kmax - 1),
                            skip_group_check=True,
                        )
                recb_sb = dp.tile([128, 512], F32, tag="recb", bufs=2)
                nc.vector.reciprocal(recb_sb[:], den_ps[:])
                nc.vector.tensor_tensor(
                    ctxN_sb[:, h, q0 : q0 + 512], ctx_ps[:], recb_sb[:], ALU.mult
                )

            ytags = [("pr", 2), ("sc", 3), ("cx", 3)]

            def outproj_group(g):
                # output projection for query chunks 4g..4g+3 (contract
                # per-core d=256 in 2 head chunks). y_ps rotates across all
                # 3 psum tags; drains alternate DVE/ACT so copies never
                # pace the PE.
                for qc in range(4 * g, 4 * g + 4):
                    for ep in range(2):
                        y_t = yp.tile([128, 1024], BF)
                        for ei in range(2):
                            eb = ep * 2 + ei
                            n = qc * 4 + eb
                            tg = ytags[n % 3]
                            y_ps = ps.tile([128, 512], F32, tag=tg[0], bufs=tg[1])
                            nc.tensor.matmul(
                                y_ps[:],
                                ctxN_sb[:, 0, qc * 128 : (qc + 1) * 128],
                                wo_sb[:, 0, eb * 512 : (eb + 1) * 512],
                                start=True,
                                stop=False,
                            )
                            nc.tensor.matmul(
                                y_ps[:],
                                ctxN_sb[:, 1, qc * 128 : (qc + 1) * 128],
                                wo_sb[:, 1, eb * 512 : (eb + 1) * 512],
                                start=False,
                                stop=True,
                            )
                            if n % 2 == 0:
                                nc.vector.tensor_copy(
                                    y_t[:, ei * 512 : (ei + 1) * 512], y_ps[:]
                                )
                            else:
                                nc.scalar.copy(
                                    y_t[:, ei * 512 : (ei + 1) * 512], y_ps[:]
                                )
                        nc.sync.dma_start(
                            yd[
                                s0 + qc * 128 : s0 + (qc + 1) * 128,
                                ep * 1024 : (ep + 1) * 1024,
                            ],
                            y_t[:],
                        )

            # Phase-separated emission: wave-major j-blocks (PE paces the
            # x DMA), last block chain-major so its psum drains stagger
            # and attention's first score matmuls find free banks.
            proj_jblock(0, wave=True)
            proj_jblock(1, wave=True)
            proj_jblock(2, wave=True)
            proj_jblock(3, wave=True)
            for qb in range(4):
                attn_unit(qb, 0, LAG=3)
                attn_unit(qb, 1, LAG=3)
            for g in range(4):
                outproj_group(g)
    _split_excess_waits(nc)
    return nc


def _pack_w(w):
    # [E, DC] -> [128, 16*DC] so row p holds w[t*128+p, :] for t=0..15
    return np.ascontiguousarray(
        w.reshape(16, 128, DC).transpose(1, 0, 2).reshape(128, 16 * DC)
    )


def kernel(**inputs):
    global LAST_RESULTS, _NC_CACHE
    x = np.asarray(inputs["x"], np.float32)
    Wq = np.asarray(inputs["Wq"], np.float32)
    bq = np.asarray(inputs["bq"], np.float32)
    Wk = np.asarray(inputs["Wk"], np.float32)
    bk = np.asarray(inputs["bk"], np.float32)
    Wv = np.asarray(inputs["Wv"], np.float32)
    bv = np.asarray(inputs["bv"], np.float32)
    Wo = np.asarray(inputs["Wo"], np.float32)
    bo = np.asarray(inputs["bo"], np.float32)

    xT = np.ascontiguousarray(x.reshape(BS, E).T).astype(bf16)
    kk = np.arange(128)[:, None]
    qq = np.arange(128)[None, :]
    misc = np.concatenate(
        [
            np.where(kk <= qq, 1.0, 0.0),
            np.ones((128, 128)),
        ],
        axis=1,
    ).astype(bf16)

    in_maps = []
    for c in range(NCORES):
        dsl = slice(c * DC, (c + 1) * DC)
        in_maps.append(
            {
                "xT": xT,
                "wq3": _pack_w((Wq[dsl].T * SCALE).astype(bf16)),
                "wk3": _pack_w(Wk[dsl].T.astype(bf16)),
                "wv3": _pack_w(Wv[dsl].T.astype(bf16)),
                "wo3": np.ascontiguousarray(
                    Wo[:, dsl].T.astype(bf16)
                    .reshape(HPC, 128, E)
                    .transpose(1, 0, 2)
                    .reshape(128, HPC * E)
                ),
                "bq": np.ascontiguousarray(
                    (bq[dsl] * SCALE).astype(np.float32).reshape(HPC, 128).T
                ),
                "misc": misc,
            }
        )

    if _NC_CACHE is None:
        _NC_CACHE = _build()
    res = run_bass_kernel_spmd(_NC_CACHE, in_maps, core_ids=list(range(NCORES)), trace=TRACE)
    LAST_RESULTS = res

    acc = None
    for r in res.results:
        yc = np.asarray(r["y"], np.float32)
        acc = yc if acc is None else acc + yc
    bo_eff = bo + bv @ Wo.T
    acc += bo_eff[None, :]
    return acc.reshape(B, S, E).astype(np.float32)
